# revision 27
# baseline (speedup 1.0000x reference)
"""Trainium2 Bass kernel for nn_Attention (dense transformer MHA block).

Reference computation (fp32):
    qkv = x @ w_qkv.T            # [B,N,3C]
    q,k,v per head; scores = q k^T / sqrt(D); attn = softmax(scores)
    o = attn @ v;  y = o @ w_proj.T + b_proj

Sharding over 8 NeuronCores (data-parallel over batch x tensor-parallel over
heads): core c -> (batch b = c//4, head group g = c%4, heads 4g..4g+3).
Each core computes q/k/v for its 4 heads over the full 2048-token sequence,
runs attention locally, and multiplies by its row-slice of w_proj, producing
a PARTIAL output [2048, 1024].  The 4 partials per batch are summed on the
host (numpy) together with the bias — no device collectives.

All matmuls run in bf16 with fp32 PSUM accumulation.  Scores are computed
transposed ([kv, q]) so exp(scores^T) feeds the A@V matmul directly; V gets an
extra ones-column so the same matmul accumulates the softmax denominator
(row 64 of the PSUM accumulator).  Softmax skips the max-subtraction (logits
are ~N(0,1); exp is safe in fp32), which is mathematically identical.

Scheduling: the PE is the critical resource (~170us of matmul streaming).
The emission keeps it gap-free:
  * qkv projection runs nchunk-outer so the first matmuls need only the
    first 1MB of x^T (DMA arrives in 512-column blocks); the k/q weights for
    head-pair 0 are produced first so attention starts ~16us in.
  * the pair-1 k/q projection and the per-qt output projection are emitted
    as PE filler INSIDE the attention unit loop (the ACT engine needs
    ~2us/group for exp vs the PE's ~1.7us of score+AV matmuls, so without
    filler the PE stalls at unit boundaries and drops to a low P-state).
  * softmax normalization: reciprocal of the denominator row on DVE, one
    DRAM-hop partition-broadcast DMA, deferred multiplies — all off the
    PE; the out-projection for query-block qt only tail-waits for qt=3.
"""

import numpy as np

B, N, C = 2, 2048, 1024
H, D = 16, 64
NCORES = 8
GROUPS = 4              # head groups (tensor-parallel)
HG = H // GROUPS        # 4 heads per core
CG = HG * D             # 256 channels per core
P = 128
KT = C // P             # 8 contraction subtiles for C=1024
KV_CHUNKS = N // P      # 16 key/value chunks of 128 rows
QT = N // 512           # 4 query tiles of 512
VB = D + 1              # v block width incl. ones column (65)
SCALE = 1.0 / float(np.sqrt(D))

_CACHED_NC = None


def _build_nc():
    from contextlib import ExitStack

    import concourse.bass as bass
    import concourse.mybir as mybir
    import concourse.tile as tile
    from concourse import bacc

    f32 = mybir.dt.float32
    bf16 = mybir.dt.bfloat16
    f16 = mybir.dt.float16
    AF = mybir.ActivationFunctionType

    nc = bacc.Bacc("TRN2", target_bir_lowering=False, debug=False,
                   num_devices=NCORES)

    # per-core inputs (host pre-sharded / pre-transposed)
    xT = nc.dram_tensor("xT", [C, N], bf16, kind="ExternalInput")
    wqkT = nc.dram_tensor("wqkT", [C, 2 * CG], bf16, kind="ExternalInput")
    wvT = nc.dram_tensor("wvT", [C, CG], bf16, kind="ExternalInput")
    wpT = nc.dram_tensor("wpT", [CG, C], bf16, kind="ExternalInput")
    yp = nc.dram_tensor("yp", [N, C], f16, kind="ExternalOutput")

    with tile.TileContext(nc) as tc:
        with ExitStack() as ctx:
            singles = ctx.enter_context(tc.tile_pool(name="singles", bufs=1))
            tmp = ctx.enter_context(tc.tile_pool(name="tmp", bufs=3))
            ps_big = ctx.enter_context(
                tc.tile_pool(name="ps_big", bufs=3, space="PSUM"))
            ps1 = ctx.enter_context(
                tc.tile_pool(name="ps1", bufs=2, space="PSUM"))
            dscratch = ctx.enter_context(
                tc.tile_pool(name="dscratch", bufs=2, space="DRAM"))

            # ---- persistent SBUF tensors -------------------------------
            xT_sb = singles.tile([P, KT, N], bf16)         # x^T (c on part)
            wqk_sb = singles.tile([P, KT, 2 * CG], bf16)   # q|k weight cols
            wv_sb = singles.tile([P, KT, CG], bf16)
            wp_sb = singles.tile([P, CG // P, C], bf16)
            qT_sb = singles.tile([P, HG // 2, N], bf16)    # q^T (d on part)
            kT_sb = singles.tile([P, HG // 2, N], bf16)    # k^T (d on part)
            v_sb = singles.tile([P, KV_CHUNKS, HG * VB], bf16)
            oT_sb = singles.tile([P, CG // P, N], bf16)    # normalized o^T

            # ---- load inputs ------------------------------------------
            xT_ap = xT.ap().rearrange("(g p) r -> p g r", p=P)
            wqk_ap = wqkT.ap().rearrange("(g p) o -> p g o", p=P)
            # k columns (mtiles 2,3) first: the projection below emits the
            # k-pair0 blocks before anything else
            nc.scalar.dma_start(wqk_sb[:, 0:4, CG:], wqk_ap[:, 0:4, CG:])
            nc.scalar.dma_start(wqk_sb[:, 4:8, CG:], wqk_ap[:, 4:8, CG:])
            nc.scalar.dma_start(wqk_sb[:, :, :CG], wqk_ap[:, :, :CG])
            # xT in 512-column blocks (block 0 split in two for latency) so
            # the nchunk-outer projection below can start after ~2.5us
            nc.sync.dma_start(xT_sb[:, 0:4, 0:512], xT_ap[:, 0:4, 0:512])
            nc.sync.dma_start(xT_sb[:, 4:8, 0:512], xT_ap[:, 4:8, 0:512])
            for nch in range(1, QT):
                nc.sync.dma_start(
                    xT_sb[:, :, nch * 512:(nch + 1) * 512],
                    xT_ap[:, :, nch * 512:(nch + 1) * 512])
            nc.scalar.dma_start(
                wv_sb[:], wvT.ap().rearrange("(g p) o -> p g o", p=P))
            nc.scalar.dma_start(
                wp_sb[:], wpT.ap().rearrange("(g p) o -> p g o", p=P))
            # whole-tile memset to 1.0; the v copies below overwrite the data
            # columns, leaving the per-head ones columns for the denominator
            nc.vector.memset(v_sb[:], 1.0)
            v_view = v_sb[:].rearrange("p c (h e) -> p c h e", e=VB)

            # ---- q^T / k^T projections, nchunk-granular ----------------
            # wqk columns: 0..CG-1 = q channels (mtiles 0,1 = pairs 0,1),
            # CG..2CG-1 = k channels (mtiles 2,3).  One qk_half emission
            # produces a 512-query block of one mtile: 8 accumulating
            # matmuls into half a [P,1024] PSUM tile plus a copy out, so
            # early blocks only need early xT DMA blocks.
            def qk_half(m, nch, pt, half):
                dst = qT_sb if m < 2 else kT_sb
                dm = m % 2
                sl = slice(half * 512, half * 512 + 512)
                for j in range(KT):
                    nc.tensor.matmul(
                        pt[:, sl],
                        wqk_sb[:, j, m * P:(m + 1) * P],
                        xT_sb[:, j, nch * 512:(nch + 1) * 512],
                        start=(j == 0), stop=(j == KT - 1))
                nc.vector.tensor_copy(
                    out=dst[:, dm, nch * 512:(nch + 1) * 512],
                    in_=pt[:, sl])

            def qk_pairblk(m0, nch0, m1, nch1):
                pt = ps_big.tile([P, 1024], f32, tag="sc",
                                 name=f"qk{m0}_{nch0}_{m1}_{nch1}")
                qk_half(m0, nch0, pt, 0)
                qk_half(m1, nch1, pt, 1)

            def v_rtile(rt):
                pt = ps_big.tile([P, 1024], f32, tag="sc", name=f"v{rt}")
                for j in range(KT):
                    nc.tensor.matmul(
                        pt[:, :CG], xT_sb[:, j, rt * P:(rt + 1) * P],
                        wv_sb[:, j, :], start=(j == 0), stop=(j == KT - 1))
                nc.vector.tensor_copy(
                    out=v_view[:, rt, :, :D],
                    in_=pt[:, :CG].rearrange("p (h d) -> p h d", d=D))

            # upfront: k pair0 over all 4 query blocks + q pair0 blocks 0,1;
            # unit 0 = (pair0, qt0) can then start.  The rest (q0 blocks
            # 2,3 and all of k1/q1) is PE filler inside unit 0's group loop
            # (k1/q1 are needed by unit 1 = (pair1, qt0)).
            qk_pairblk(2, 0, 2, 1)
            qk_pairblk(2, 2, 2, 3)
            qk_pairblk(0, 0, 0, 1)
            qk_fill = [(3, 0), (3, 1), (3, 2), (3, 3), (1, 0), (1, 1)]

            # ---- attention: software-pipelined emission ----------------
            # Units are (pair, qt), qt-major so each 512-row block of the
            # output projection can be emitted as PE filler soon after its
            # two units finish.  Within the global stream, the A@V matmuls
            # for group t are emitted AFTER the score matmuls of group t+1:
            # the PE is in-order, so this one-group skew keeps it from
            # stalling on the exp (ACT) results.
            GROUP = 2  # kv chunks per exp batch (PSUM tile = 2 banks)
            NGRP = KV_CHUNKS // GROUP

            pending_recips = []
            pending_muls = []

            def flush_muls(keep=0):
                while len(pending_muls) > keep:
                    pending_muls.pop(0)()

            def flush_recips(keep=0):
                while len(pending_recips) > keep:
                    pending_recips.pop(0)()

            def normalize_pair(o_acc_pair, pair, qt, last=False):
                # Stage both unnormalized accumulators to SBUF immediately
                # so the PSUM banks free for the next unit's A@V, and start
                # the denominator's DRAM-hop spread to [128,4] (reciprocal
                # there costs 172ns on DVE vs 3.3us on [1,512]).  The
                # reciprocal+broadcast are deferred ONE unit and the final
                # multiplies TWO units, so the in-order DVE stream never
                # waits on any DMA round trip.  DMAs ride the sync/gpsimd
                # queues — never scalar, whose in-order sequencer would
                # stall the exp stream on the DMA semaphore waits.
                for hx, po in ((0, 0), (1, D)):
                    # the last unit's chain is the kernel tail: ACT is done
                    # by then, so hx1 may use the fast scalar HW-DGE queue
                    # instead of the ~1.3us-per-DMA gpsimd SW-DGE
                    eng = nc.sync if hx == 0 else (
                        nc.scalar if last else nc.gpsimd)
                    ou = tmp.tile([VB, 512], f32, tag="ou", bufs=6,
                                  name=f"ou{pair}_{qt}_{hx}")
                    nc.vector.tensor_copy(out=ou[:],
                                          in_=o_acc_pair[hx][:VB])
                    den_d = dscratch.tile([1, 512], f32, tag="dend",
                                          name=f"dend{pair}_{qt}_{hx}")
                    eng.dma_start(den_d[:], ou[D:D + 1, :])
                    den_p = tmp.tile([P, 4], f32, tag="denp", bufs=4,
                                     name=f"denp{pair}_{qt}_{hx}")
                    eng.dma_start(
                        den_p[:],
                        den_d[:].rearrange("o (j p) -> p (o j)", p=P))

                    def recip(den_p=den_p, ou=ou, po=po, pair=pair, qt=qt,
                              hx=hx, eng=eng):
                        rec_p = tmp.tile([P, 4], f32, tag="recp", bufs=4,
                                         name=f"recp{pair}_{qt}_{hx}")
                        nc.vector.reciprocal(out=rec_p[:], in_=den_p[:])
                        rec_d = dscratch.tile([1, 512], f32, tag="recd",
                                              name=f"recd{pair}_{qt}_{hx}")
                        eng.dma_start(
                            rec_d[:].rearrange("o (j p) -> p (o j)", p=P),
                            rec_p[:])
                        bc_sb = tmp.tile([D, 512], f32, tag="bcsb", bufs=4,
                                         name=f"bcsb{pair}_{qt}_{hx}")
                        rec_bcast = bass.AP(
                            tensor=rec_d.tensor, offset=rec_d.offset,
                            ap=[[0, D]] + [list(p) for p in rec_d.ap[1:]])
                        eng.dma_start(bc_sb[:], rec_bcast)

                        def mul(ou=ou, bc_sb=bc_sb, po=po, pair=pair, qt=qt):
                            nc.vector.tensor_mul(
                                out=oT_sb[po:po + D, pair,
                                          qt * 512:(qt + 1) * 512],
                                in0=ou[:D, :], in1=bc_sb[:])
                        pending_muls.append(mul)
                    pending_recips.append(recip)

            def proj_mt(mt):
                # partial output projection for rows [mt*128, mt*128+128)
                pp = ps_big.tile([P, 1024], f32, tag="sc", name=f"pp{mt}")
                for nh in range(2):
                    for j in range(CG // P):
                        nc.tensor.matmul(
                            pp[:, nh * 512:nh * 512 + 512],
                            oT_sb[:, j, mt * P:(mt + 1) * P],
                            wp_sb[:, j, nh * 512:(nh + 1) * 512],
                            start=(j == 0), stop=(j == CG // P - 1))
                ysb = tmp.tile([P, 1024], f16, tag="ysb", bufs=4,
                               name=f"ysb{mt}")
                nc.vector.tensor_copy(out=ysb[:], in_=pp[:])
                # final stores gate kernel end: use HW-DGE queues (ACT is
                # done by then, so scalar is safe)
                if mt >= 12:
                    eng = (nc.sync, nc.scalar)[mt % 2]
                else:
                    eng = (nc.sync, nc.gpsimd)[mt % 2]
                eng.dma_start(yp.ap()[mt * P:(mt + 1) * P, :], ysb[:])

            units = [(pair, qt) for qt in range(QT) for pair in range(HG // 2)]
            o_accs_u = {}
            pending = None      # (u, g) whose A@V is not yet emitted

            def emit_av(u, g, exs):
                pair, qt = units[u]
                hA, hB = 2 * pair, 2 * pair + 1
                for i in range(GROUP):
                    r = g * GROUP + i
                    for hx, h in ((0, hA), (1, hB)):
                        nc.tensor.matmul(
                            o_accs_u[u][hx][:VB, :],
                            v_sb[:, r, h * VB:(h + 1) * VB],
                            exs[hx][:, i * 512:i * 512 + 512],
                            start=(r == 0), stop=(r == KV_CHUNKS - 1))
                if g == NGRP - 1:
                    normalize_pair(o_accs_u[u], pair, qt,
                                   last=(u == len(units) - 1))
                    # emit the previous unit's reciprocal chain (its den
                    # spread arrived a unit ago) and the unit-before-that's
                    # multiplies (their broadcasts completed a unit ago):
                    # no DVE instruction ever waits on a DMA round trip
                    flush_recips(keep=2)
                    flush_muls(keep=2)
                    del o_accs_u[u]

            # PE filler per (unit, group): unit 0 carries the remaining
            # q/k projection blocks; units 2..7 carry the out-projection
            # for the query block completed two units earlier.
            filler = {}
            for g in range(len(qk_fill) // 2):
                a0, b0 = qk_fill[2 * g]
                a1, b1 = qk_fill[2 * g + 1]
                filler[(0, g)] = [lambda a0=a0, b0=b0, a1=a1, b1=b1:
                                  qk_pairblk(a0, b0, a1, b1)]
            # units 2/3 carry the q blocks needed from unit 4 on (their
            # projection filler slot is empty: muls for query block qt are
            # emitted at the end of unit 2qt+3 under the two-unit deferral,
            # so qt0's projection is only safe from unit 4)
            filler[(2, 3)] = [lambda: qk_pairblk(0, 2, 1, 2)]
            filler[(3, 3)] = [lambda: qk_pairblk(0, 3, 1, 3)]
            for u in range(4, 8):
                qt_done = (u - 4) // 2
                mt0 = qt_done * 4 + 2 * (u % 2)
                filler[(u, 3)] = [lambda mt=mt0: proj_mt(mt)]
                filler[(u, 6)] = [lambda mt=mt0 + 1: proj_mt(mt)]

            for u, (pair, qt) in enumerate(units):
                qs = slice(qt * 512, (qt + 1) * 512)
                o_accs_u[u] = [ps1.tile([P, 512], f32, tag="ps1",
                                        name=f"oacc{pair}_{qt}_{i}")
                               for i in range(2)]
                for g in range(NGRP):
                    if u == 0:
                        v_rtile(2 * g)
                        v_rtile(2 * g + 1)
                    scs = [ps_big.tile([P, 1024], f32, tag="sc",
                                       name=f"sc{pair}_{qt}_{g}_{i}")
                           for i in range(2)]
                    for i in range(GROUP):
                        r = g * GROUP + i
                        for hx, po in ((0, 0), (1, D)):
                            nc.tensor.matmul(
                                scs[hx][:, i * 512:i * 512 + 512],
                                kT_sb[po:po + D, pair, r * P:(r + 1) * P],
                                qT_sb[po:po + D, pair, qs],
                                start=True, stop=True)
                    exs = []
                    for hx in range(2):
                        ex = tmp.tile([P, 1024], bf16, tag="ex", bufs=6,
                                      name=f"ex{pair}_{qt}_{g}_{hx}")
                        nc.scalar.activation(
                            ex[:], scs[hx][:], AF.Exp, scale=SCALE)
                        exs.append(ex)
                    if pending is not None:
                        emit_av(*pending)
                    for f in filler.pop((u, g), ()):
                        f()
                    pending = (u, g, exs)
            emit_av(*pending)
            # tail: qt3-pair0's muls (unit 6, still deferred) flush first —
            # their broadcast launched at unit 7's end, so they clear while
            # proj 8/9 runs and only unit 7's muls gate the last blocks.
            # qt2's projection blocks stagger across both DMA segments of
            # unit 7's normalize chain.
            flush_muls()
            proj_mt(8)
            proj_mt(9)
            flush_recips()
            proj_mt(10)
            proj_mt(11)
            flush_muls()
            for mt in (12, 13, 14, 15):
                proj_mt(mt)

    nc.compile()
    return nc


def _host_prep(x, w_qkv, w_proj, b_proj):
    import ml_dtypes
    bf16 = ml_dtypes.bfloat16
    wqkvT = np.ascontiguousarray(w_qkv.T).astype(bf16)   # [C, 3C]
    wpT_full = np.ascontiguousarray(w_proj.T).astype(bf16)  # [C(in), C(out)]
    in_maps = []
    for c in range(NCORES):
        b, g = divmod(c, GROUPS)
        qcols = wqkvT[:, CG * g:CG * (g + 1)]
        kcols = wqkvT[:, C + CG * g:C + CG * (g + 1)]
        vcols = wqkvT[:, 2 * C + CG * g:2 * C + CG * (g + 1)]
        wqk = np.ascontiguousarray(np.concatenate([qcols, kcols], axis=1))
        wv = np.ascontiguousarray(vcols)
        wp = np.ascontiguousarray(wpT_full[CG * g:CG * (g + 1), :])
        xTv = np.ascontiguousarray(x[b].T).astype(bf16)
        in_maps.append({"xT": xTv, "wqkT": wqk, "wvT": wv, "wpT": wp})
    return in_maps


def run(inputs, trace=False, nc=None):
    """Build (or reuse) the program, run on 8 cores, return (y, results)."""
    global _CACHED_NC
    from concourse.bass_utils import run_bass_kernel_spmd
    if nc is None:
        if _CACHED_NC is None:
            _CACHED_NC = _build_nc()
        nc = _CACHED_NC
    in_maps = _host_prep(**inputs)
    res = run_bass_kernel_spmd(nc, in_maps, core_ids=list(range(NCORES)),
                               trace=trace)
    bias = np.asarray(inputs["b_proj"], np.float32)
    out = np.empty((B, N, C), np.float32)
    for b in range(B):
        acc = res.results[b * GROUPS]["yp"].astype(np.float32)
        for g in range(1, GROUPS):
            acc = acc + res.results[b * GROUPS + g]["yp"]
        out[b] = acc + bias
    return out, res


def kernel(x, w_qkv, w_proj, b_proj):
    out, _ = run({"x": np.asarray(x), "w_qkv": np.asarray(w_qkv),
                  "w_proj": np.asarray(w_proj), "b_proj": np.asarray(b_proj)})
    return out


# revision 29
# speedup vs baseline: 1.1531x; 1.1531x over previous
"""Trainium2 Bass kernel for nn_Attention (dense transformer MHA block).

Reference computation (fp32):
    qkv = x @ w_qkv.T            # [B,N,3C]
    q,k,v per head; scores = q k^T / sqrt(D); attn = softmax(scores)
    o = attn @ v;  y = o @ w_proj.T + b_proj

Sharding over 8 NeuronCores (data-parallel over batch x tensor-parallel over
heads): core c -> (batch b = c//4, head group g = c%4, heads 4g..4g+3).
Each core computes q/k/v for its 4 heads over the full 2048-token sequence,
runs attention locally, and multiplies by its row-slice of w_proj, producing
a PARTIAL output [2048, 1024].  The 4 partials per batch are summed on the
host (numpy) together with the bias — no device collectives.

All matmuls run in bf16 with fp32 PSUM accumulation.  Scores are computed
transposed ([kv, q]) so exp(scores^T) feeds the A@V matmul directly; V gets an
extra ones-column so the same matmul accumulates the softmax denominator
(row 64 of the PSUM accumulator).  Softmax skips the max-subtraction (logits
are ~N(0,1); exp is safe in fp32), which is mathematically identical.

Scheduling: the PE is the critical resource (~170us of matmul streaming).
The emission keeps it gap-free:
  * qkv projection runs nchunk-outer so the first matmuls need only the
    first 1MB of x^T (DMA arrives in 512-column blocks); the k/q weights for
    head-pair 0 are produced first so attention starts ~16us in.
  * the pair-1 k/q projection and the per-qt output projection are emitted
    as PE filler INSIDE the attention unit loop (the ACT engine needs
    ~2us/group for exp vs the PE's ~1.7us of score+AV matmuls, so without
    filler the PE stalls at unit boundaries and drops to a low P-state).
  * softmax normalization: reciprocal of the denominator row on DVE, one
    DRAM-hop partition-broadcast DMA, deferred multiplies — all off the
    PE; the out-projection for query-block qt only tail-waits for qt=3.
"""

import numpy as np

B, N, C = 2, 2048, 1024
H, D = 16, 64
NCORES = 8
GROUPS = 4              # head groups (tensor-parallel)
HG = H // GROUPS        # 4 heads per core
CG = HG * D             # 256 channels per core
P = 128
KT = C // P             # 8 contraction subtiles for C=1024
KV_CHUNKS = N // P      # 16 key/value chunks of 128 rows
QT = N // 512           # 4 query tiles of 512
VB = D + 1              # v block width incl. ones column (65)
SCALE = 1.0 / float(np.sqrt(D))

_CACHED_NC = None


def _build_nc():
    from contextlib import ExitStack

    import concourse.bass as bass
    import concourse.mybir as mybir
    import concourse.tile as tile
    from concourse import bacc

    f32 = mybir.dt.float32
    bf16 = mybir.dt.bfloat16
    f16 = mybir.dt.float16
    AF = mybir.ActivationFunctionType

    nc = bacc.Bacc("TRN2", target_bir_lowering=False, debug=False,
                   num_devices=NCORES)

    # per-core inputs (host pre-sharded / pre-transposed)
    xT = nc.dram_tensor("xT", [C, N], bf16, kind="ExternalInput")
    wqkT = nc.dram_tensor("wqkT", [C, 2 * CG], bf16, kind="ExternalInput")
    wvT = nc.dram_tensor("wvT", [C, CG], bf16, kind="ExternalInput")
    wpT = nc.dram_tensor("wpT", [CG, C], bf16, kind="ExternalInput")
    yp = nc.dram_tensor("yp", [N, C], f16, kind="ExternalOutput")

    with tile.TileContext(nc) as tc:
        with ExitStack() as ctx:
            singles = ctx.enter_context(tc.tile_pool(name="singles", bufs=1))
            tmp = ctx.enter_context(tc.tile_pool(name="tmp", bufs=3))
            ps_big = ctx.enter_context(
                tc.tile_pool(name="ps_big", bufs=3, space="PSUM"))
            ps1 = ctx.enter_context(
                tc.tile_pool(name="ps1", bufs=2, space="PSUM"))
            dscratch = ctx.enter_context(
                tc.tile_pool(name="dscratch", bufs=2, space="DRAM"))

            # ---- persistent SBUF tensors -------------------------------
            xT_sb = singles.tile([P, KT, N], bf16)         # x^T (c on part)
            wqk_sb = singles.tile([P, KT, 2 * CG], bf16)   # q|k weight cols
            wv_sb = singles.tile([P, KT, CG], bf16)
            wp_sb = singles.tile([P, CG // P, C], bf16)
            qT_sb = singles.tile([P, HG // 2, N], bf16)    # q^T (d on part)
            kT_sb = singles.tile([P, HG // 2, N], bf16)    # k^T (d on part)
            v_sb = singles.tile([P, KV_CHUNKS, HG * VB], bf16)
            oT_sb = singles.tile([P, CG // P, N], bf16)    # normalized o^T

            # ---- load inputs ------------------------------------------
            xT_ap = xT.ap().rearrange("(g p) r -> p g r", p=P)
            wqk_ap = wqkT.ap().rearrange("(g p) o -> p g o", p=P)
            # k columns (mtiles 2,3) first: the projection below emits the
            # k-pair0 blocks before anything else
            nc.scalar.dma_start(wqk_sb[:, 0:4, CG:], wqk_ap[:, 0:4, CG:])
            nc.scalar.dma_start(wqk_sb[:, 4:8, CG:], wqk_ap[:, 4:8, CG:])
            nc.scalar.dma_start(wqk_sb[:, :, :CG], wqk_ap[:, :, :CG])
            # xT in 512-column blocks (block 0 split in two for latency) so
            # the nchunk-outer projection below can start after ~2.5us
            nc.sync.dma_start(xT_sb[:, 0:4, 0:512], xT_ap[:, 0:4, 0:512])
            nc.sync.dma_start(xT_sb[:, 4:8, 0:512], xT_ap[:, 4:8, 0:512])
            for nch in range(1, QT):
                nc.sync.dma_start(
                    xT_sb[:, :, nch * 512:(nch + 1) * 512],
                    xT_ap[:, :, nch * 512:(nch + 1) * 512])
            nc.scalar.dma_start(
                wv_sb[:], wvT.ap().rearrange("(g p) o -> p g o", p=P))
            nc.scalar.dma_start(
                wp_sb[:], wpT.ap().rearrange("(g p) o -> p g o", p=P))
            # whole-tile memset to 1.0; the v copies below overwrite the data
            # columns, leaving the per-head ones columns for the denominator
            nc.vector.memset(v_sb[:], 1.0)
            v_view = v_sb[:].rearrange("p c (h e) -> p c h e", e=VB)

            # ---- q^T / k^T projections, nchunk-granular ----------------
            # wqk columns: 0..CG-1 = q channels (mtiles 0,1 = pairs 0,1),
            # CG..2CG-1 = k channels (mtiles 2,3).  One qk_half emission
            # produces a 512-query block of one mtile: 8 accumulating
            # matmuls into half a [P,1024] PSUM tile plus a copy out, so
            # early blocks only need early xT DMA blocks.
            def qk_half(m, nch, pt, half):
                dst = qT_sb if m < 2 else kT_sb
                dm = m % 2
                sl = slice(half * 512, half * 512 + 512)
                for j in range(KT):
                    nc.tensor.matmul(
                        pt[:, sl],
                        wqk_sb[:, j, m * P:(m + 1) * P],
                        xT_sb[:, j, nch * 512:(nch + 1) * 512],
                        start=(j == 0), stop=(j == KT - 1))
                nc.vector.tensor_copy(
                    out=dst[:, dm, nch * 512:(nch + 1) * 512],
                    in_=pt[:, sl])

            def qk_pairblk(m0, nch0, m1, nch1):
                pt = ps_big.tile([P, 1024], f32, tag="sc",
                                 name=f"qk{m0}_{nch0}_{m1}_{nch1}")
                qk_half(m0, nch0, pt, 0)
                qk_half(m1, nch1, pt, 1)

            def v_rtile(rt):
                pt = ps_big.tile([P, 1024], f32, tag="sc", name=f"v{rt}")
                for j in range(KT):
                    nc.tensor.matmul(
                        pt[:, :CG], xT_sb[:, j, rt * P:(rt + 1) * P],
                        wv_sb[:, j, :], start=(j == 0), stop=(j == KT - 1))
                nc.vector.tensor_copy(
                    out=v_view[:, rt, :, :D],
                    in_=pt[:, :CG].rearrange("p (h d) -> p h d", d=D))

            # upfront: k pair0 over all 4 query blocks + q pair0 blocks 0,1;
            # unit 0 = (pair0, qt0) can then start.  The rest (q0 blocks
            # 2,3 and all of k1/q1) is PE filler inside unit 0's group loop
            # (k1/q1 are needed by unit 1 = (pair1, qt0)).
            qk_pairblk(2, 0, 2, 1)
            qk_pairblk(2, 2, 2, 3)
            qk_pairblk(0, 0, 0, 1)
            qk_fill = [(3, 0), (3, 1), (3, 2), (3, 3), (1, 0), (1, 1)]

            # ---- attention: software-pipelined emission ----------------
            # Units are (pair, qt), qt-major so each 512-row block of the
            # output projection can be emitted as PE filler soon after its
            # two units finish.  Within the global stream, the A@V matmuls
            # for group t are emitted AFTER the score matmuls of group t+1:
            # the PE is in-order, so this one-group skew keeps it from
            # stalling on the exp (ACT) results.
            GROUP = 2  # kv chunks per exp batch (PSUM tile = 2 banks)
            NGRP = KV_CHUNKS // GROUP

            pending_recips = []
            pending_muls = []

            def flush_muls(keep=0):
                while len(pending_muls) > keep:
                    pending_muls.pop(0)()

            def flush_recips(keep=0):
                while len(pending_recips) > keep:
                    pending_recips.pop(0)()

            def normalize_pair(o_acc_pair, pair, qt, last=False):
                # Stage both unnormalized accumulators to SBUF immediately
                # so the PSUM banks free for the next unit's A@V, and start
                # the denominator's DRAM-hop spread to [128,4] (reciprocal
                # there costs 172ns on DVE vs 3.3us on [1,512]).  The
                # reciprocal+broadcast are deferred ONE unit and the final
                # multiplies TWO units, so the in-order DVE stream never
                # waits on any DMA round trip.  DMAs ride the sync/gpsimd
                # queues — never scalar, whose in-order sequencer would
                # stall the exp stream on the DMA semaphore waits.
                for hx, po in ((0, 0), (1, D)):
                    # the last unit's chain is the kernel tail: ACT is done
                    # by then, so hx1 may use the fast scalar HW-DGE queue
                    # instead of the ~1.3us-per-DMA gpsimd SW-DGE
                    eng = nc.sync if hx == 0 else (
                        nc.scalar if last else nc.gpsimd)
                    ou = tmp.tile([VB, 512], f32, tag="ou", bufs=6,
                                  name=f"ou{pair}_{qt}_{hx}")
                    nc.vector.tensor_copy(out=ou[:],
                                          in_=o_acc_pair[hx][:VB])
                    den_d = dscratch.tile([1, 512], f32, tag="dend",
                                          name=f"dend{pair}_{qt}_{hx}")
                    eng.dma_start(den_d[:], ou[D:D + 1, :])
                    den_p = tmp.tile([P, 4], f32, tag="denp", bufs=4,
                                     name=f"denp{pair}_{qt}_{hx}")
                    eng.dma_start(
                        den_p[:],
                        den_d[:].rearrange("o (j p) -> p (o j)", p=P))

                    def recip(den_p=den_p, ou=ou, po=po, pair=pair, qt=qt,
                              hx=hx, eng=eng):
                        rec_p = tmp.tile([P, 4], f32, tag="recp", bufs=4,
                                         name=f"recp{pair}_{qt}_{hx}")
                        nc.vector.reciprocal(out=rec_p[:], in_=den_p[:])
                        rec_d = dscratch.tile([1, 512], f32, tag="recd",
                                              name=f"recd{pair}_{qt}_{hx}")
                        eng.dma_start(
                            rec_d[:].rearrange("o (j p) -> p (o j)", p=P),
                            rec_p[:])
                        bc_sb = tmp.tile([D, 512], f32, tag="bcsb", bufs=4,
                                         name=f"bcsb{pair}_{qt}_{hx}")
                        rec_bcast = bass.AP(
                            tensor=rec_d.tensor, offset=rec_d.offset,
                            ap=[[0, D]] + [list(p) for p in rec_d.ap[1:]])
                        eng.dma_start(bc_sb[:], rec_bcast)

                        def mul(ou=ou, bc_sb=bc_sb, po=po, pair=pair, qt=qt):
                            nc.vector.tensor_mul(
                                out=oT_sb[po:po + D, pair,
                                          qt * 512:(qt + 1) * 512],
                                in0=ou[:D, :], in1=bc_sb[:])
                        pending_muls.append(mul)
                    pending_recips.append(recip)

            def proj_mt(mt):
                # partial output projection for rows [mt*128, mt*128+128)
                pp = ps_big.tile([P, 1024], f32, tag="sc", name=f"pp{mt}")
                for nh in range(2):
                    for j in range(CG // P):
                        nc.tensor.matmul(
                            pp[:, nh * 512:nh * 512 + 512],
                            oT_sb[:, j, mt * P:(mt + 1) * P],
                            wp_sb[:, j, nh * 512:(nh + 1) * 512],
                            start=(j == 0), stop=(j == CG // P - 1))
                ysb = tmp.tile([P, 1024], f16, tag="ysb", bufs=4,
                               name=f"ysb{mt}")
                nc.vector.tensor_copy(out=ysb[:], in_=pp[:])
                # final stores gate kernel end: use HW-DGE queues (ACT is
                # done by then, so scalar is safe)
                if mt >= 12:
                    eng = (nc.sync, nc.scalar)[mt % 2]
                else:
                    eng = (nc.sync, nc.gpsimd)[mt % 2]
                eng.dma_start(yp.ap()[mt * P:(mt + 1) * P, :], ysb[:])

            units = [(pair, qt) for qt in range(QT) for pair in range(HG // 2)]
            o_accs_u = {}
            pending = None      # (u, g) whose A@V is not yet emitted

            def emit_av(u, g, exs):
                pair, qt = units[u]
                hA, hB = 2 * pair, 2 * pair + 1
                for i in range(GROUP):
                    r = g * GROUP + i
                    for hx, h in ((0, hA), (1, hB)):
                        nc.tensor.matmul(
                            o_accs_u[u][hx][:VB, :],
                            v_sb[:, r, h * VB:(h + 1) * VB],
                            exs[hx][:, i * 512:i * 512 + 512],
                            start=(r == 0), stop=(r == KV_CHUNKS - 1))
                if g == NGRP - 1:
                    normalize_pair(o_accs_u[u], pair, qt,
                                   last=(u == len(units) - 1))
                    # emit the previous unit's reciprocal chain (its den
                    # spread arrived a unit ago) and the unit-before-that's
                    # multiplies (their broadcasts completed a unit ago):
                    # no DVE instruction ever waits on a DMA round trip
                    flush_recips(keep=2)
                    flush_muls(keep=2)
                    del o_accs_u[u]

            # PE filler per (unit, group): unit 0 carries the remaining
            # q/k projection blocks; units 2..7 carry the out-projection
            # for the query block completed two units earlier.
            filler = {}
            for g in range(len(qk_fill) // 2):
                a0, b0 = qk_fill[2 * g]
                a1, b1 = qk_fill[2 * g + 1]
                filler[(0, g)] = [lambda a0=a0, b0=b0, a1=a1, b1=b1:
                                  qk_pairblk(a0, b0, a1, b1)]
            # units 2/3 carry the q blocks needed from unit 4 on (their
            # projection filler slot is empty: muls for query block qt are
            # emitted at the end of unit 2qt+3 under the two-unit deferral,
            # so qt0's projection is only safe from unit 4)
            filler[(2, 3)] = [lambda: qk_pairblk(0, 2, 1, 2)]
            filler[(3, 3)] = [lambda: qk_pairblk(0, 3, 1, 3)]
            for u in range(4, 8):
                qt_done = (u - 4) // 2
                mt0 = qt_done * 4 + 2 * (u % 2)
                filler[(u, 3)] = [lambda mt=mt0: proj_mt(mt)]
                filler[(u, 6)] = [lambda mt=mt0 + 1: proj_mt(mt)]

            for u, (pair, qt) in enumerate(units):
                qs = slice(qt * 512, (qt + 1) * 512)
                o_accs_u[u] = [ps1.tile([P, 512], f32, tag="ps1",
                                        name=f"oacc{pair}_{qt}_{i}")
                               for i in range(2)]
                for g in range(NGRP):
                    if u == 0:
                        v_rtile(2 * g)
                        v_rtile(2 * g + 1)
                    scs = [ps_big.tile([P, 1024], f32, tag="sc",
                                       name=f"sc{pair}_{qt}_{g}_{i}")
                           for i in range(2)]
                    for i in range(GROUP):
                        r = g * GROUP + i
                        for hx, po in ((0, 0), (1, D)):
                            nc.tensor.matmul(
                                scs[hx][:, i * 512:i * 512 + 512],
                                kT_sb[po:po + D, pair, r * P:(r + 1) * P],
                                qT_sb[po:po + D, pair, qs],
                                start=True, stop=True)
                    exs = []
                    for hx in range(2):
                        ex = tmp.tile([P, 1024], bf16, tag="ex", bufs=10,
                                      name=f"ex{pair}_{qt}_{g}_{hx}")
                        nc.scalar.activation(
                            ex[:], scs[hx][:], AF.Exp, scale=SCALE)
                        exs.append(ex)
                    if pending is not None:
                        emit_av(*pending)
                    for f in filler.pop((u, g), ()):
                        f()
                    pending = (u, g, exs)
            emit_av(*pending)
            # tail: stagger qt2's projection blocks so the PE covers BOTH
            # DMA segments of unit 7's normalize chain — the denominator
            # spread flies under proj 8/9 (so the deferred reciprocal
            # never waits), the reciprocal broadcast under proj 10/11,
            # and the final flush enables qt3's projection
            proj_mt(8)
            proj_mt(9)
            flush_recips()
            proj_mt(10)
            proj_mt(11)
            flush_muls()
            for mt in (12, 13, 14, 15):
                proj_mt(mt)

    nc.compile()
    return nc


def _host_prep(x, w_qkv, w_proj, b_proj):
    import ml_dtypes
    bf16 = ml_dtypes.bfloat16
    wqkvT = np.ascontiguousarray(w_qkv.T).astype(bf16)   # [C, 3C]
    wpT_full = np.ascontiguousarray(w_proj.T).astype(bf16)  # [C(in), C(out)]
    in_maps = []
    for c in range(NCORES):
        b, g = divmod(c, GROUPS)
        qcols = wqkvT[:, CG * g:CG * (g + 1)]
        kcols = wqkvT[:, C + CG * g:C + CG * (g + 1)]
        vcols = wqkvT[:, 2 * C + CG * g:2 * C + CG * (g + 1)]
        wqk = np.ascontiguousarray(np.concatenate([qcols, kcols], axis=1))
        wv = np.ascontiguousarray(vcols)
        wp = np.ascontiguousarray(wpT_full[CG * g:CG * (g + 1), :])
        xTv = np.ascontiguousarray(x[b].T).astype(bf16)
        in_maps.append({"xT": xTv, "wqkT": wqk, "wvT": wv, "wpT": wp})
    return in_maps


def run(inputs, trace=False, nc=None):
    """Build (or reuse) the program, run on 8 cores, return (y, results)."""
    global _CACHED_NC
    from concourse.bass_utils import run_bass_kernel_spmd
    if nc is None:
        if _CACHED_NC is None:
            _CACHED_NC = _build_nc()
        nc = _CACHED_NC
    in_maps = _host_prep(**inputs)
    res = run_bass_kernel_spmd(nc, in_maps, core_ids=list(range(NCORES)),
                               trace=trace)
    bias = np.asarray(inputs["b_proj"], np.float32)
    out = np.empty((B, N, C), np.float32)
    for b in range(B):
        acc = res.results[b * GROUPS]["yp"].astype(np.float32)
        for g in range(1, GROUPS):
            acc = acc + res.results[b * GROUPS + g]["yp"]
        out[b] = acc + bias
    return out, res


def kernel(x, w_qkv, w_proj, b_proj):
    out, _ = run({"x": np.asarray(x), "w_qkv": np.asarray(w_qkv),
                  "w_proj": np.asarray(w_proj), "b_proj": np.asarray(b_proj)})
    return out


# revision 30
# speedup vs baseline: 1.1855x; 1.0281x over previous
"""Trainium2 Bass kernel for nn_Attention (dense transformer MHA block).

Reference computation (fp32):
    qkv = x @ w_qkv.T            # [B,N,3C]
    q,k,v per head; scores = q k^T / sqrt(D); attn = softmax(scores)
    o = attn @ v;  y = o @ w_proj.T + b_proj

Sharding over 8 NeuronCores (data-parallel over batch x tensor-parallel over
heads): core c -> (batch b = c//4, head group g = c%4, heads 4g..4g+3).
Each core computes q/k/v for its 4 heads over the full 2048-token sequence,
runs attention locally, and multiplies by its row-slice of w_proj, producing
a PARTIAL output [2048, 1024].  The 4 partials per batch are summed on the
host (numpy) together with the bias — no device collectives.

All matmuls run in bf16 with fp32 PSUM accumulation.  Scores are computed
transposed ([kv, q]) so exp(scores^T) feeds the A@V matmul directly; V gets an
extra ones-column so the same matmul accumulates the softmax denominator
(row 64 of the PSUM accumulator).  Softmax skips the max-subtraction (logits
are ~N(0,1); exp is safe in fp32), which is mathematically identical.

Scheduling: the PE is the critical resource (~170us of matmul streaming).
The emission keeps it gap-free:
  * qkv projection runs nchunk-outer so the first matmuls need only the
    first 1MB of x^T (DMA arrives in 512-column blocks); the k/q weights for
    head-pair 0 are produced first so attention starts ~16us in.
  * the pair-1 k/q projection and the per-qt output projection are emitted
    as PE filler INSIDE the attention unit loop (the ACT engine needs
    ~2us/group for exp vs the PE's ~1.7us of score+AV matmuls, so without
    filler the PE stalls at unit boundaries and drops to a low P-state).
  * softmax normalization: reciprocal of the denominator row on DVE, one
    DRAM-hop partition-broadcast DMA, deferred multiplies — all off the
    PE; the out-projection for query-block qt only tail-waits for qt=3.
"""

import numpy as np

B, N, C = 2, 2048, 1024
H, D = 16, 64
NCORES = 8
GROUPS = 4              # head groups (tensor-parallel)
HG = H // GROUPS        # 4 heads per core
CG = HG * D             # 256 channels per core
P = 128
KT = C // P             # 8 contraction subtiles for C=1024
KV_CHUNKS = N // P      # 16 key/value chunks of 128 rows
QT = N // 512           # 4 query tiles of 512
VB = D + 1              # v block width incl. ones column (65)
SCALE = 1.0 / float(np.sqrt(D))

_CACHED_NC = None


def _build_nc():
    from contextlib import ExitStack

    import concourse.bass as bass
    import concourse.mybir as mybir
    import concourse.tile as tile
    from concourse import bacc

    f32 = mybir.dt.float32
    bf16 = mybir.dt.bfloat16
    f16 = mybir.dt.float16
    AF = mybir.ActivationFunctionType

    nc = bacc.Bacc("TRN2", target_bir_lowering=False, debug=False,
                   num_devices=NCORES)

    # per-core inputs (host pre-sharded / pre-transposed)
    xT = nc.dram_tensor("xT", [C, N], bf16, kind="ExternalInput")
    wqkT = nc.dram_tensor("wqkT", [C, 2 * CG], bf16, kind="ExternalInput")
    wvT = nc.dram_tensor("wvT", [C, CG], bf16, kind="ExternalInput")
    wpT = nc.dram_tensor("wpT", [CG, C], bf16, kind="ExternalInput")
    yp = nc.dram_tensor("yp", [N, C], f16, kind="ExternalOutput")

    with tile.TileContext(nc) as tc:
        with ExitStack() as ctx:
            singles = ctx.enter_context(tc.tile_pool(name="singles", bufs=1))
            tmp = ctx.enter_context(tc.tile_pool(name="tmp", bufs=3))
            ps_big = ctx.enter_context(
                tc.tile_pool(name="ps_big", bufs=3, space="PSUM"))
            ps1 = ctx.enter_context(
                tc.tile_pool(name="ps1", bufs=2, space="PSUM"))
            dscratch = ctx.enter_context(
                tc.tile_pool(name="dscratch", bufs=4, space="DRAM"))

            # ---- persistent SBUF tensors -------------------------------
            xT_sb = singles.tile([P, KT, N], bf16)         # x^T (c on part)
            wqk_sb = singles.tile([P, KT, 2 * CG], bf16)   # q|k weight cols
            wv_sb = singles.tile([P, KT, CG], bf16)
            wp_sb = singles.tile([P, CG // P, C], bf16)
            qT_sb = singles.tile([P, HG // 2, N], bf16)    # q^T (d on part)
            kT_sb = singles.tile([P, HG // 2, N], bf16)    # k^T (d on part)
            v_sb = singles.tile([P, KV_CHUNKS, HG * VB], bf16)
            oT_sb = singles.tile([P, CG // P, N], bf16)    # normalized o^T

            # ---- load inputs ------------------------------------------
            xT_ap = xT.ap().rearrange("(g p) r -> p g r", p=P)
            wqk_ap = wqkT.ap().rearrange("(g p) o -> p g o", p=P)
            # k columns (mtiles 2,3) first: the projection below emits the
            # k-pair0 blocks before anything else
            nc.scalar.dma_start(wqk_sb[:, 0:4, CG:], wqk_ap[:, 0:4, CG:])
            nc.scalar.dma_start(wqk_sb[:, 4:8, CG:], wqk_ap[:, 4:8, CG:])
            nc.scalar.dma_start(wqk_sb[:, :, :CG], wqk_ap[:, :, :CG])
            # xT in 512-column blocks (block 0 split in two for latency) so
            # the nchunk-outer projection below can start after ~2.5us
            nc.sync.dma_start(xT_sb[:, 0:4, 0:512], xT_ap[:, 0:4, 0:512])
            nc.sync.dma_start(xT_sb[:, 4:8, 0:512], xT_ap[:, 4:8, 0:512])
            for nch in range(1, QT):
                nc.sync.dma_start(
                    xT_sb[:, :, nch * 512:(nch + 1) * 512],
                    xT_ap[:, :, nch * 512:(nch + 1) * 512])
            nc.scalar.dma_start(
                wv_sb[:], wvT.ap().rearrange("(g p) o -> p g o", p=P))
            nc.scalar.dma_start(
                wp_sb[:], wpT.ap().rearrange("(g p) o -> p g o", p=P))
            # whole-tile memset to 1.0; the v copies below overwrite the data
            # columns, leaving the per-head ones columns for the denominator
            nc.vector.memset(v_sb[:], 1.0)
            v_view = v_sb[:].rearrange("p c (h e) -> p c h e", e=VB)

            # ---- q^T / k^T projections, nchunk-granular ----------------
            # wqk columns: 0..CG-1 = q channels (mtiles 0,1 = pairs 0,1),
            # CG..2CG-1 = k channels (mtiles 2,3).  One qk_half emission
            # produces a 512-query block of one mtile: 8 accumulating
            # matmuls into half a [P,1024] PSUM tile plus a copy out, so
            # early blocks only need early xT DMA blocks.
            def qk_half(m, nch, pt, half):
                dst = qT_sb if m < 2 else kT_sb
                dm = m % 2
                sl = slice(half * 512, half * 512 + 512)
                for j in range(KT):
                    nc.tensor.matmul(
                        pt[:, sl],
                        wqk_sb[:, j, m * P:(m + 1) * P],
                        xT_sb[:, j, nch * 512:(nch + 1) * 512],
                        start=(j == 0), stop=(j == KT - 1))
                nc.vector.tensor_copy(
                    out=dst[:, dm, nch * 512:(nch + 1) * 512],
                    in_=pt[:, sl])

            def qk_pairblk(m0, nch0, m1, nch1):
                pt = ps_big.tile([P, 1024], f32, tag="sc",
                                 name=f"qk{m0}_{nch0}_{m1}_{nch1}")
                qk_half(m0, nch0, pt, 0)
                qk_half(m1, nch1, pt, 1)

            def v_rtile(rt):
                pt = ps_big.tile([P, 1024], f32, tag="sc", name=f"v{rt}")
                for j in range(KT):
                    nc.tensor.matmul(
                        pt[:, :CG], xT_sb[:, j, rt * P:(rt + 1) * P],
                        wv_sb[:, j, :], start=(j == 0), stop=(j == KT - 1))
                nc.vector.tensor_copy(
                    out=v_view[:, rt, :, :D],
                    in_=pt[:, :CG].rearrange("p (h d) -> p h d", d=D))

            # upfront: k pair0 over all 4 query blocks + q pair0 blocks 0,1;
            # unit 0 = (pair0, qt0) can then start.  The rest (q0 blocks
            # 2,3 and all of k1/q1) is PE filler inside unit 0's group loop
            # (k1/q1 are needed by unit 1 = (pair1, qt0)).
            qk_pairblk(2, 0, 2, 1)
            qk_pairblk(2, 2, 2, 3)
            qk_pairblk(0, 0, 0, 1)
            qk_fill = [(3, 0), (3, 1), (3, 2), (3, 3), (1, 0), (1, 1)]

            # ---- attention: software-pipelined emission ----------------
            # Units are (pair, qt), qt-major so each 512-row block of the
            # output projection can be emitted as PE filler soon after its
            # two units finish.  Within the global stream, the A@V matmuls
            # for group t are emitted AFTER the score matmuls of group t+1:
            # the PE is in-order, so this one-group skew keeps it from
            # stalling on the exp (ACT) results.
            GROUP = 2  # kv chunks per exp batch (PSUM tile = 2 banks)
            NGRP = KV_CHUNKS // GROUP

            pending_recips = []
            pending_muls = []

            def flush_muls(keep=0):
                while len(pending_muls) > keep:
                    pending_muls.pop(0)()

            def flush_recips(keep=0):
                while len(pending_recips) > keep:
                    pending_recips.pop(0)()

            def normalize_pair(o_acc_pair, pair, qt, last=False):
                # Stage both unnormalized accumulators to SBUF immediately
                # so the PSUM banks free for the next unit's A@V, and start
                # the denominator's DRAM-hop spread to [128,4] (reciprocal
                # there costs 172ns on DVE vs 3.3us on [1,512]).  The
                # reciprocal+broadcast are deferred ONE unit and the final
                # multiplies TWO units, so the in-order DVE stream never
                # waits on any DMA round trip.  DMAs ride the sync/gpsimd
                # queues — never scalar, whose in-order sequencer would
                # stall the exp stream on the DMA semaphore waits.
                for hx, po in ((0, 0), (1, D)):
                    # the last unit's chain is the kernel tail: ACT is done
                    # by then, so hx1 may use the fast scalar HW-DGE queue
                    # instead of the ~1.3us-per-DMA gpsimd SW-DGE
                    eng = nc.sync if hx == 0 else (
                        nc.scalar if last else nc.gpsimd)
                    ou = tmp.tile([VB, 512], f32, tag="ou", bufs=8,
                                  name=f"ou{pair}_{qt}_{hx}")
                    nc.vector.tensor_copy(out=ou[:],
                                          in_=o_acc_pair[hx][:VB])
                    den_d = dscratch.tile([1, 512], f32, tag="dend",
                                          name=f"dend{pair}_{qt}_{hx}")
                    eng.dma_start(den_d[:], ou[D:D + 1, :])
                    den_p = tmp.tile([P, 4], f32, tag="denp", bufs=6,
                                     name=f"denp{pair}_{qt}_{hx}")
                    eng.dma_start(
                        den_p[:],
                        den_d[:].rearrange("o (j p) -> p (o j)", p=P))

                    def recip(den_p=den_p, ou=ou, po=po, pair=pair, qt=qt,
                              hx=hx, eng=eng):
                        rec_p = tmp.tile([P, 4], f32, tag="recp", bufs=6,
                                         name=f"recp{pair}_{qt}_{hx}")
                        nc.vector.reciprocal(out=rec_p[:], in_=den_p[:])
                        rec_d = dscratch.tile([1, 512], f32, tag="recd",
                                              name=f"recd{pair}_{qt}_{hx}")
                        eng.dma_start(
                            rec_d[:].rearrange("o (j p) -> p (o j)", p=P),
                            rec_p[:])
                        bc_sb = tmp.tile([D, 512], f32, tag="bcsb", bufs=6,
                                         name=f"bcsb{pair}_{qt}_{hx}")
                        rec_bcast = bass.AP(
                            tensor=rec_d.tensor, offset=rec_d.offset,
                            ap=[[0, D]] + [list(p) for p in rec_d.ap[1:]])
                        eng.dma_start(bc_sb[:], rec_bcast)

                        def mul(ou=ou, bc_sb=bc_sb, po=po, pair=pair, qt=qt):
                            nc.vector.tensor_mul(
                                out=oT_sb[po:po + D, pair,
                                          qt * 512:(qt + 1) * 512],
                                in0=ou[:D, :], in1=bc_sb[:])
                        pending_muls.append(mul)
                    pending_recips.append(recip)

            def proj_mt(mt):
                # partial output projection for rows [mt*128, mt*128+128)
                pp = ps_big.tile([P, 1024], f32, tag="sc", name=f"pp{mt}")
                for nh in range(2):
                    for j in range(CG // P):
                        nc.tensor.matmul(
                            pp[:, nh * 512:nh * 512 + 512],
                            oT_sb[:, j, mt * P:(mt + 1) * P],
                            wp_sb[:, j, nh * 512:(nh + 1) * 512],
                            start=(j == 0), stop=(j == CG // P - 1))
                ysb = tmp.tile([P, 1024], f16, tag="ysb", bufs=6,
                               name=f"ysb{mt}")
                nc.vector.tensor_copy(out=ysb[:], in_=pp[:])
                # final stores gate kernel end: use HW-DGE queues (ACT is
                # done by then, so scalar is safe)
                if mt >= 12:
                    eng = (nc.sync, nc.scalar)[mt % 2]
                else:
                    eng = (nc.sync, nc.gpsimd)[mt % 2]
                eng.dma_start(yp.ap()[mt * P:(mt + 1) * P, :], ysb[:])

            units = [(pair, qt) for qt in range(QT) for pair in range(HG // 2)]
            o_accs_u = {}
            pending = None      # (u, g) whose A@V is not yet emitted

            def emit_av(u, g, exs):
                pair, qt = units[u]
                hA, hB = 2 * pair, 2 * pair + 1
                for i in range(GROUP):
                    r = g * GROUP + i
                    for hx, h in ((0, hA), (1, hB)):
                        nc.tensor.matmul(
                            o_accs_u[u][hx][:VB, :],
                            v_sb[:, r, h * VB:(h + 1) * VB],
                            exs[hx][:, i * 512:i * 512 + 512],
                            start=(r == 0), stop=(r == KV_CHUNKS - 1))
                if g == NGRP - 1:
                    normalize_pair(o_accs_u[u], pair, qt,
                                   last=(u == len(units) - 1))
                    # emit the previous unit's reciprocal chain (its den
                    # spread arrived a unit ago) and the unit-before-that's
                    # multiplies (their broadcasts completed a unit ago):
                    # no DVE instruction ever waits on a DMA round trip
                    flush_recips(keep=2)
                    flush_muls(keep=2)
                    del o_accs_u[u]

            # PE filler per (unit, group): unit 0 carries the remaining
            # q/k projection blocks; units 2..7 carry the out-projection
            # for the query block completed two units earlier.
            filler = {}
            for g in range(len(qk_fill) // 2):
                a0, b0 = qk_fill[2 * g]
                a1, b1 = qk_fill[2 * g + 1]
                filler[(0, g)] = [lambda a0=a0, b0=b0, a1=a1, b1=b1:
                                  qk_pairblk(a0, b0, a1, b1)]
            # units 2/3 carry the q blocks needed from unit 4 on (their
            # projection filler slot is empty: muls for query block qt are
            # emitted at the end of unit 2qt+3 under the two-unit deferral,
            # so qt0's projection is only safe from unit 4)
            filler[(2, 3)] = [lambda: qk_pairblk(0, 2, 1, 2)]
            filler[(3, 3)] = [lambda: qk_pairblk(0, 3, 1, 3)]
            for u in range(4, 8):
                qt_done = (u - 4) // 2
                mt0 = qt_done * 4 + 2 * (u % 2)
                filler[(u, 3)] = [lambda mt=mt0: proj_mt(mt)]
                filler[(u, 6)] = [lambda mt=mt0 + 1: proj_mt(mt)]

            for u, (pair, qt) in enumerate(units):
                qs = slice(qt * 512, (qt + 1) * 512)
                o_accs_u[u] = [ps1.tile([P, 512], f32, tag="ps1",
                                        name=f"oacc{pair}_{qt}_{i}")
                               for i in range(2)]
                for g in range(NGRP):
                    if u == 0:
                        v_rtile(2 * g)
                        v_rtile(2 * g + 1)
                    scs = [ps_big.tile([P, 1024], f32, tag="sc",
                                       name=f"sc{pair}_{qt}_{g}_{i}")
                           for i in range(2)]
                    for i in range(GROUP):
                        r = g * GROUP + i
                        for hx, po in ((0, 0), (1, D)):
                            nc.tensor.matmul(
                                scs[hx][:, i * 512:i * 512 + 512],
                                kT_sb[po:po + D, pair, r * P:(r + 1) * P],
                                qT_sb[po:po + D, pair, qs],
                                start=True, stop=True)
                    exs = []
                    for hx in range(2):
                        ex = tmp.tile([P, 1024], bf16, tag="ex", bufs=10,
                                      name=f"ex{pair}_{qt}_{g}_{hx}")
                        nc.scalar.activation(
                            ex[:], scs[hx][:], AF.Exp, scale=SCALE)
                        exs.append(ex)
                    if pending is not None:
                        emit_av(*pending)
                    for f in filler.pop((u, g), ()):
                        f()
                    pending = (u, g, exs)
            emit_av(*pending)
            # tail: stagger qt2's projection blocks so the PE covers BOTH
            # DMA segments of unit 7's normalize chain — the denominator
            # spread flies under proj 8/9 (so the deferred reciprocal
            # never waits), the reciprocal broadcast under proj 10/11,
            # and the final flush enables qt3's projection
            proj_mt(8)
            proj_mt(9)
            flush_recips()
            proj_mt(10)
            proj_mt(11)
            flush_muls()
            for mt in (12, 13, 14, 15):
                proj_mt(mt)

    nc.compile()
    return nc


def _host_prep(x, w_qkv, w_proj, b_proj):
    import ml_dtypes
    bf16 = ml_dtypes.bfloat16
    wqkvT = np.ascontiguousarray(w_qkv.T).astype(bf16)   # [C, 3C]
    wpT_full = np.ascontiguousarray(w_proj.T).astype(bf16)  # [C(in), C(out)]
    in_maps = []
    for c in range(NCORES):
        b, g = divmod(c, GROUPS)
        qcols = wqkvT[:, CG * g:CG * (g + 1)]
        kcols = wqkvT[:, C + CG * g:C + CG * (g + 1)]
        vcols = wqkvT[:, 2 * C + CG * g:2 * C + CG * (g + 1)]
        wqk = np.ascontiguousarray(np.concatenate([qcols, kcols], axis=1))
        wv = np.ascontiguousarray(vcols)
        wp = np.ascontiguousarray(wpT_full[CG * g:CG * (g + 1), :])
        xTv = np.ascontiguousarray(x[b].T).astype(bf16)
        in_maps.append({"xT": xTv, "wqkT": wqk, "wvT": wv, "wpT": wp})
    return in_maps


def run(inputs, trace=False, nc=None):
    """Build (or reuse) the program, run on 8 cores, return (y, results)."""
    global _CACHED_NC
    from concourse.bass_utils import run_bass_kernel_spmd
    if nc is None:
        if _CACHED_NC is None:
            _CACHED_NC = _build_nc()
        nc = _CACHED_NC
    in_maps = _host_prep(**inputs)
    res = run_bass_kernel_spmd(nc, in_maps, core_ids=list(range(NCORES)),
                               trace=trace)
    bias = np.asarray(inputs["b_proj"], np.float32)
    out = np.empty((B, N, C), np.float32)
    for b in range(B):
        acc = res.results[b * GROUPS]["yp"].astype(np.float32)
        for g in range(1, GROUPS):
            acc = acc + res.results[b * GROUPS + g]["yp"]
        out[b] = acc + bias
    return out, res


def kernel(x, w_qkv, w_proj, b_proj):
    out, _ = run({"x": np.asarray(x), "w_qkv": np.asarray(w_qkv),
                  "w_proj": np.asarray(w_proj), "b_proj": np.asarray(b_proj)})
    return out
